# revision 7
# baseline (speedup 1.0000x reference)
"""Trainium2 Bass kernel for GraphSpectralFilterLayer.

Reference computation:
    h = x @ W.T                                  [4096, 128]
    mean = attention.mean()                      (global scalar)
    a = where(att > mean, att, -9e15); LeakyReLU(0.2); softmax(dim=0)
    a = where(drop_mask, a / 0.4, 0)
    out = (a @ h) reshaped (4,4096,128)->(4096, 512)

Exact simplifications (att ~ U[0,1), mean > 0):
    - kept values are positive so LeakyReLU is identity on them; dropped
      values give exp(0.2 * -9e15) == 0 exactly in f32. Hence
      v = exp(att) * (att > mean), softmax = v / colsum(v)  (shift-free
      exp is safe: att in [0,1)).
    - softmax denominator does NOT include the dropout mask.
    - (v / colsum) @ h == v @ (h * (1/(0.4*colsum))[:, None]) -- fold
      normalization + dropout scale into the tiny h matrix.

Sharding: rows of the [16384, 4096] attention matrix across 8 cores
(2048 rows each). softmax(dim=0) needs only a [128, 32] AllReduce of
column sums; the global mean is a [1, 1] AllReduce. attention/mask are
passed host-transposed so tiles land in SBUF with the contraction dim
(j, graph nodes) on partitions -- no on-chip transposes. Output is
produced transposed [128, 2048] per core, un-transposed on host.
"""

import sys

sys.path.insert(0, "/opt/trn_rl_repo")

import numpy as np

from concourse import bass, bacc, tile, mybir
from concourse.bass_utils import run_bass_kernel_spmd

N = 4096          # graph nodes (columns of attention)
CN = 16384        # C * N rows of attention
OUT_F = 128
IN_F = 512
N_CORES = 8
ROWS = CN // N_CORES          # 2048 local attention rows (i)
JT = N // 128                 # 32 j-tiles
JC = 8                        # j-tiles of v cached in SBUF (of 32)
NCN = float(CN) * float(N)    # mean divisor

F32 = mybir.dt.float32
U8 = mybir.dt.uint8
AX = mybir.AxisListType
OP = mybir.AluOpType
AF = mybir.ActivationFunctionType


def _build():
    nc = bacc.Bacc("TRN2", target_bir_lowering=False, debug=False,
                   num_devices=N_CORES)

    attT = nc.dram_tensor("attT", [N, ROWS], F32, kind="ExternalInput")
    maskT = nc.dram_tensor("maskT", [N, ROWS], U8, kind="ExternalInput")
    xT = nc.dram_tensor("xT", [IN_F, N], F32, kind="ExternalInput")
    wT = nc.dram_tensor("wT", [IN_F, OUT_F], F32, kind="ExternalInput")
    outT = nc.dram_tensor("outT", [OUT_F, ROWS], F32, kind="ExternalOutput")

    with tile.TileContext(nc) as tc:
        with tc.tile_pool(name="persist", bufs=1) as persist, \
             tc.tile_pool(name="dram", bufs=1, space="DRAM") as dram:
            # ---- persistent SBUF state ----
            vcache = persist.tile([128, JC * ROWS], F32)   # cached v tiles
            h = persist.tile([128, JT * OUT_F], F32)       # h, per j-tile
            acc = persist.tile([128, JT], F32)             # mean partials
            cs = persist.tile([128, JT], F32)              # colsum partials
            ones_ff = persist.tile([128, 128], F32)
            tot = persist.tile([128, 1], F32)
            gsum = persist.tile([128, 1], F32)
            mean_bc = persist.tile([128, 1], F32)
            csum = persist.tile([128, JT], F32)
            rcs2 = persist.tile([128, JT], F32)
            nc.vector.memset(ones_ff[:, :], 1.0)

            # collective bounce buffers (DRAM, non-I/O)
            cc_mean_in = dram.tile([128, 1], F32)
            cc_mean_out = dram.tile([128, 1], F32)
            cc_cs_in = dram.tile([128, JT], F32)
            cc_cs_out = dram.tile([128, JT], F32)

            # ---- h = x @ W.T  (from host-transposed xT, wT) ----
            with tc.tile_pool(name="xw", bufs=1) as xw, \
                 tc.tile_pool(name="hps", bufs=2, space="PSUM") as hps:
                wt_t = xw.tile([128, 4 * OUT_F], F32, tag="wt")
                xt_ts = []
                for ct in range(4):
                    nc.sync.dma_start(
                        out=wt_t[:, ct * OUT_F:(ct + 1) * OUT_F],
                        in_=wT[ct * 128:(ct + 1) * 128, :])
                    xt_t = xw.tile([128, N], F32, tag=f"xt{ct}")
                    nc.sync.dma_start(out=xt_t[:, :],
                                      in_=xT[ct * 128:(ct + 1) * 128, :])
                    xt_ts.append(xt_t)
                for jt in range(JT):
                    ps = hps.tile([128, OUT_F], F32, tag="hps")
                    for ct in range(4):
                        nc.tensor.matmul(
                            ps[:, :],
                            lhsT=xt_ts[ct][:, jt * 128:(jt + 1) * 128],
                            rhs=wt_t[:, ct * OUT_F:(ct + 1) * OUT_F],
                            start=(ct == 0), stop=(ct == 3))
                    nc.scalar.copy(h[:, jt * OUT_F:(jt + 1) * OUT_F],
                                   ps[:, :])

            # ---- P1: global mean ----
            with tc.tile_pool(name="p1", bufs=3) as p1:
                for jt in range(JT):
                    a_t = p1.tile([128, ROWS], F32, tag="a1")
                    nc.sync.dma_start(out=a_t[:, :],
                                      in_=attT[jt * 128:(jt + 1) * 128, :])
                    nc.vector.tensor_reduce(out=acc[:, jt:jt + 1],
                                            in_=a_t[:, :], axis=AX.X,
                                            op=OP.add)
            with tc.tile_pool(name="p1b", bufs=1, space="PSUM") as p1b:
                nc.vector.tensor_reduce(out=tot[:, :], in_=acc[:, :],
                                        axis=AX.X, op=OP.add)
                nc.sync.dma_start(out=cc_mean_in[:, :], in_=tot[:, :])
                nc.gpsimd.collective_compute(
                    "AllReduce", OP.add,
                    replica_groups=[list(range(N_CORES))],
                    ins=[cc_mean_in[:, :].opt()],
                    outs=[cc_mean_out[:, :].opt()])
                nc.gpsimd.dma_start(out=gsum[:, :], in_=cc_mean_out[:, :])
                ps_bc = p1b.tile([128, 1], F32, tag="bc")
                nc.tensor.matmul(ps_bc[:, :], lhsT=ones_ff[:, :],
                                 rhs=gsum[:, :], start=True, stop=True)
                nc.vector.tensor_scalar(out=mean_bc[:, :], in0=ps_bc[:, :],
                                        scalar1=1.0 / NCN, scalar2=None,
                                        op0=OP.mult)

            # ---- P2: v = exp(att) * (att > mean); column sums ----
            with tc.tile_pool(name="p2", bufs=2) as p2:
                for jt in range(JT):
                    a_t = p2.tile([128, ROWS], F32, tag="a2")
                    nc.sync.dma_start(out=a_t[:, :],
                                      in_=attT[jt * 128:(jt + 1) * 128, :])
                    e_t = p2.tile([128, ROWS], F32, tag="e2")
                    nc.scalar.activation(e_t[:, :], a_t[:, :], AF.Exp)
                    g_t = p2.tile([128, ROWS], F32, tag="g2")
                    nc.vector.tensor_scalar(out=g_t[:, :], in0=a_t[:, :],
                                            scalar1=mean_bc[:, 0:1],
                                            scalar2=None, op0=OP.is_gt)
                    if jt < JC:
                        v_dst = vcache[:, jt * ROWS:(jt + 1) * ROWS]
                    else:
                        v_t = p2.tile([128, ROWS], F32, tag="v2")
                        v_dst = v_t[:, :]
                    nc.vector.tensor_tensor(out=v_dst, in0=g_t[:, :],
                                            in1=e_t[:, :], op=OP.mult)
                    nc.vector.tensor_reduce(out=cs[:, jt:jt + 1], in_=v_dst,
                                            axis=AX.X, op=OP.add)

            # ---- colsum AllReduce; fold 1/(0.4*colsum) into h ----
            nc.sync.dma_start(out=cc_cs_in[:, :], in_=cs[:, :])
            nc.gpsimd.collective_compute(
                "AllReduce", OP.add,
                replica_groups=[list(range(N_CORES))],
                ins=[cc_cs_in[:, :].opt()],
                outs=[cc_cs_out[:, :].opt()])
            nc.sync.dma_start(out=csum[:, :], in_=cc_cs_out[:, :])
            nc.vector.tensor_scalar(out=rcs2[:, :], in0=csum[:, :],
                                    scalar1=0.4, scalar2=None, op0=OP.mult)
            nc.vector.reciprocal(rcs2[:, :], rcs2[:, :])
            for jt in range(JT):
                nc.vector.tensor_scalar(
                    out=h[:, jt * OUT_F:(jt + 1) * OUT_F],
                    in0=h[:, jt * OUT_F:(jt + 1) * OUT_F],
                    scalar1=rcs2[:, jt:jt + 1], scalar2=None, op0=OP.mult)

            # ---- P3: vm = v * mask; outT[f, i] += h_s[jt].T @ vm ----
            with tc.tile_pool(name="p3", bufs=2) as p3, \
                 tc.tile_pool(name="p3r", bufs=1) as p3r, \
                 tc.tile_pool(name="ops", bufs=1, space="PSUM") as ops:
                ps_o = []
                for ic in range(4):
                    ps_oc = ops.tile([128, 512], F32, tag=f"o{ic}")
                    ps_o.append(ps_oc)
                for jt in range(JT):
                    if jt < JC:
                        v_src = vcache[:, jt * ROWS:(jt + 1) * ROWS]
                    else:
                        a_t = p3r.tile([128, ROWS], F32, tag="a3")
                        nc.sync.dma_start(
                            out=a_t[:, :],
                            in_=attT[jt * 128:(jt + 1) * 128, :])
                        e_t = p3r.tile([128, ROWS], F32, tag="e3")
                        nc.scalar.activation(e_t[:, :], a_t[:, :], AF.Exp)
                        g_t = p3r.tile([128, ROWS], F32, tag="g3")
                        nc.vector.tensor_scalar(out=g_t[:, :], in0=a_t[:, :],
                                                scalar1=mean_bc[:, 0:1],
                                                scalar2=None, op0=OP.is_gt)
                        v_t = p3r.tile([128, ROWS], F32, tag="v3")
                        nc.vector.tensor_tensor(out=v_t[:, :], in0=g_t[:, :],
                                                in1=e_t[:, :], op=OP.mult)
                        v_src = v_t[:, :]
                    m_t = p3.tile([128, ROWS], U8, tag="m3")
                    nc.sync.dma_start(out=m_t[:, :],
                                      in_=maskT[jt * 128:(jt + 1) * 128, :])
                    mf_t = p3.tile([128, ROWS], F32, tag="mf3")
                    nc.gpsimd.tensor_copy(mf_t[:, :], m_t[:, :])
                    vm_t = p3.tile([128, ROWS], F32, tag="vm3")
                    nc.gpsimd.tensor_tensor(out=vm_t[:, :], in0=v_src,
                                            in1=mf_t[:, :], op=OP.mult)
                    for ic in range(4):
                        nc.tensor.matmul(
                            ps_o[ic][:, :],
                            lhsT=h[:, jt * OUT_F:(jt + 1) * OUT_F],
                            rhs=vm_t[:, ic * 512:(ic + 1) * 512],
                            start=(jt == 0), stop=(jt == JT - 1))
                for ic in range(4):
                    o_t = p3.tile([128, 512], F32, tag="osb")
                    nc.scalar.copy(o_t[:, :], ps_o[ic][:, :])
                    nc.sync.dma_start(out=outT[:, ic * 512:(ic + 1) * 512],
                                      in_=o_t[:, :])
    nc.compile()
    return nc


def kernel(x, attention, W, drop_mask):
    attT = np.ascontiguousarray(attention.T)           # [4096, 16384] f32
    maskT = np.ascontiguousarray(
        drop_mask.astype(np.uint8, copy=False).T)      # [4096, 16384] u8
    xT = np.ascontiguousarray(x.T)                     # [512, 4096]
    wT = np.ascontiguousarray(W.T)                     # [512, 128]

    nc = _build()
    in_maps = []
    for c in range(N_CORES):
        sl = slice(c * ROWS, (c + 1) * ROWS)
        in_maps.append({
            "attT": np.ascontiguousarray(attT[:, sl]),
            "maskT": np.ascontiguousarray(maskT[:, sl]),
            "xT": xT,
            "wT": wT,
        })
    res = run_bass_kernel_spmd(nc, in_maps, core_ids=list(range(N_CORES)))
    global LAST_EXEC_NS
    LAST_EXEC_NS = res.exec_time_ns or res.mean_exec_time_ns
    h_prime = np.concatenate(
        [res.results[c]["outT"].T for c in range(N_CORES)], axis=0)
    out = (h_prime.reshape(4, N, OUT_F).transpose(1, 0, 2)
           .reshape(N, 4 * OUT_F))
    return np.ascontiguousarray(out)


if __name__ == "__main__":
    rng = np.random.default_rng(0)
    x = rng.standard_normal((N, IN_F), dtype=np.float32)
    att = rng.random((CN, N), dtype=np.float32)
    W = (rng.standard_normal((OUT_F, IN_F), dtype=np.float32)
         / np.sqrt(IN_F)).astype(np.float32)
    dm = rng.integers(0, 2, size=(CN, N)).astype(bool)
    out = kernel(x=x, attention=att, W=W, drop_mask=dm)
    print("kernel out", out.shape, out.dtype, float(np.abs(out).max()))


# revision 8
# speedup vs baseline: 140.7272x; 140.7272x over previous
"""Trainium2 Bass kernel for GraphSpectralFilterLayer.

Reference computation:
    h = x @ W.T                                  [4096, 128]
    mean = attention.mean()                      (global scalar)
    a = where(att > mean, att, -9e15); LeakyReLU(0.2); softmax(dim=0)
    a = where(drop_mask, a / 0.4, 0)
    out = (a @ h) reshaped (4,4096,128)->(4096, 512)

Exact simplifications (att ~ U[0,1), mean > 0):
    - kept values are positive so LeakyReLU is identity on them; dropped
      values give exp(0.2 * -9e15) == 0 exactly in f32. Hence
      v = exp(att) * (att > mean), softmax = v / colsum(v)  (shift-free
      exp is safe: att in [0,1)).
    - softmax denominator does NOT include the dropout mask.
    - (v / colsum) @ h == v @ (h * (1/(0.4*colsum))[:, None]) -- fold
      normalization + dropout scale into the tiny h matrix.

Sharding: rows of the [16384, 4096] attention matrix across 8 cores
(2048 rows each). softmax(dim=0) needs only a [128, 32] AllReduce of
column sums; the global mean is a [1, 1] AllReduce. attention/mask are
passed host-transposed so tiles land in SBUF with the contraction dim
(j, graph nodes) on partitions -- no on-chip transposes. Output is
produced transposed [128, 2048] per core, un-transposed on host.
"""

import sys

sys.path.insert(0, "/opt/trn_rl_repo")

import numpy as np

from concourse import bass, bacc, tile, mybir
from concourse.bass_utils import run_bass_kernel_spmd

N = 4096          # graph nodes (columns of attention)
CN = 16384        # C * N rows of attention
OUT_F = 128
IN_F = 512
N_CORES = 8
ROWS = CN // N_CORES          # 2048 local attention rows (i)
JT = N // 128                 # 32 j-tiles
JC = 8                        # j-tiles of v cached in SBUF (of 32)
NCN = float(CN) * float(N)    # mean divisor

F32 = mybir.dt.float32
U8 = mybir.dt.uint8
AX = mybir.AxisListType
OP = mybir.AluOpType
AF = mybir.ActivationFunctionType


def _build():
    nc = bacc.Bacc("TRN2", target_bir_lowering=False, debug=False,
                   num_devices=N_CORES)

    attT = nc.dram_tensor("attT", [N, ROWS], F32, kind="ExternalInput")
    maskT = nc.dram_tensor("maskT", [N, ROWS], U8, kind="ExternalInput")
    xT = nc.dram_tensor("xT", [IN_F, N], F32, kind="ExternalInput")
    wT = nc.dram_tensor("wT", [IN_F, OUT_F], F32, kind="ExternalInput")
    outT = nc.dram_tensor("outT", [OUT_F, ROWS], F32, kind="ExternalOutput")

    with tile.TileContext(nc) as tc:
        with tc.tile_pool(name="persist", bufs=1) as persist, \
             tc.tile_pool(name="dram", bufs=1, space="DRAM") as dram:
            # ---- persistent SBUF state ----
            vcache = persist.tile([128, JC * ROWS], F32)   # cached v tiles
            h = persist.tile([128, JT * OUT_F], F32)       # h, per j-tile
            acc = persist.tile([128, JT], F32)             # mean partials
            cs = persist.tile([128, JT], F32)              # colsum partials
            ones_ff = persist.tile([128, 128], F32)
            tot = persist.tile([128, 1], F32)
            gsum = persist.tile([128, 1], F32)
            mean_bc = persist.tile([128, 1], F32)
            csum = persist.tile([128, JT], F32)
            rcs2 = persist.tile([128, JT], F32)
            nc.vector.memset(ones_ff[:, :], 1.0)

            # collective bounce buffers (DRAM, non-I/O)
            cc_mean_in = dram.tile([128, 1], F32)
            cc_mean_out = dram.tile([128, 1], F32)
            cc_cs_in = dram.tile([128, JT], F32)
            cc_cs_out = dram.tile([128, JT], F32)

            # ---- h = x @ W.T  (from host-transposed xT, wT) ----
            with tc.tile_pool(name="xw", bufs=1) as xw, \
                 tc.tile_pool(name="hps", bufs=2, space="PSUM") as hps:
                wt_t = xw.tile([128, 4 * OUT_F], F32, tag="wt")
                xt_ts = []
                for ct in range(4):
                    nc.sync.dma_start(
                        out=wt_t[:, ct * OUT_F:(ct + 1) * OUT_F],
                        in_=wT[ct * 128:(ct + 1) * 128, :])
                    xt_t = xw.tile([128, N], F32, tag=f"xt{ct}")
                    nc.sync.dma_start(out=xt_t[:, :],
                                      in_=xT[ct * 128:(ct + 1) * 128, :])
                    xt_ts.append(xt_t)
                for jt in range(JT):
                    ps = hps.tile([128, OUT_F], F32, tag="hps")
                    for ct in range(4):
                        nc.tensor.matmul(
                            ps[:, :],
                            lhsT=xt_ts[ct][:, jt * 128:(jt + 1) * 128],
                            rhs=wt_t[:, ct * OUT_F:(ct + 1) * OUT_F],
                            start=(ct == 0), stop=(ct == 3))
                    nc.scalar.copy(h[:, jt * OUT_F:(jt + 1) * OUT_F],
                                   ps[:, :])

            # ---- P1: global mean ----
            with tc.tile_pool(name="p1", bufs=4) as p1:
                for jt in range(JT):
                    a_t = p1.tile([128, ROWS], F32, tag="a1")
                    nc.sync.dma_start(out=a_t[:, :],
                                      in_=attT[jt * 128:(jt + 1) * 128, :])
                    nc.vector.tensor_reduce(out=acc[:, jt:jt + 1],
                                            in_=a_t[:, :], axis=AX.X,
                                            op=OP.add)
            with tc.tile_pool(name="p1b", bufs=1, space="PSUM") as p1b:
                nc.vector.tensor_reduce(out=tot[:, :], in_=acc[:, :],
                                        axis=AX.X, op=OP.add)
                nc.sync.dma_start(out=cc_mean_in[:, :], in_=tot[:, :])
                nc.gpsimd.collective_compute(
                    "AllReduce", OP.add,
                    replica_groups=[list(range(N_CORES))],
                    ins=[cc_mean_in[:, :].opt()],
                    outs=[cc_mean_out[:, :].opt()])
                nc.gpsimd.dma_start(out=gsum[:, :], in_=cc_mean_out[:, :])
                ps_bc = p1b.tile([128, 1], F32, tag="bc")
                nc.tensor.matmul(ps_bc[:, :], lhsT=ones_ff[:, :],
                                 rhs=gsum[:, :], start=True, stop=True)
                nc.vector.tensor_scalar(out=mean_bc[:, :], in0=ps_bc[:, :],
                                        scalar1=1.0 / NCN, scalar2=None,
                                        op0=OP.mult)

            # ---- P2: v = exp(att) * (att > mean); column sums ----
            with tc.tile_pool(name="p2", bufs=3) as p2:
                for jt in range(JT):
                    a_t = p2.tile([128, ROWS], F32, tag="a2")
                    nc.sync.dma_start(out=a_t[:, :],
                                      in_=attT[jt * 128:(jt + 1) * 128, :])
                    e_t = p2.tile([128, ROWS], F32, tag="e2")
                    nc.scalar.activation(e_t[:, :], a_t[:, :], AF.Exp)
                    g_t = p2.tile([128, ROWS], F32, tag="g2")
                    nc.vector.tensor_scalar(out=g_t[:, :], in0=a_t[:, :],
                                            scalar1=mean_bc[:, 0:1],
                                            scalar2=None, op0=OP.is_gt)
                    if jt < JC:
                        v_dst = vcache[:, jt * ROWS:(jt + 1) * ROWS]
                    else:
                        v_t = p2.tile([128, ROWS], F32, tag="v2")
                        v_dst = v_t[:, :]
                    nc.vector.tensor_tensor(out=v_dst, in0=g_t[:, :],
                                            in1=e_t[:, :], op=OP.mult)
                    nc.vector.tensor_reduce(out=cs[:, jt:jt + 1], in_=v_dst,
                                            axis=AX.X, op=OP.add)

            # ---- colsum AllReduce; fold 1/(0.4*colsum) into h ----
            nc.sync.dma_start(out=cc_cs_in[:, :], in_=cs[:, :])
            nc.gpsimd.collective_compute(
                "AllReduce", OP.add,
                replica_groups=[list(range(N_CORES))],
                ins=[cc_cs_in[:, :].opt()],
                outs=[cc_cs_out[:, :].opt()])
            nc.sync.dma_start(out=csum[:, :], in_=cc_cs_out[:, :])
            nc.vector.tensor_scalar(out=rcs2[:, :], in0=csum[:, :],
                                    scalar1=0.4, scalar2=None, op0=OP.mult)
            nc.vector.reciprocal(rcs2[:, :], rcs2[:, :])
            for jt in range(JT):
                nc.vector.tensor_scalar(
                    out=h[:, jt * OUT_F:(jt + 1) * OUT_F],
                    in0=h[:, jt * OUT_F:(jt + 1) * OUT_F],
                    scalar1=rcs2[:, jt:jt + 1], scalar2=None, op0=OP.mult)

            # ---- P3: vm = v * mask; outT[f, i] += h_s[jt].T @ vm ----
            with tc.tile_pool(name="p3", bufs=3) as p3, \
                 tc.tile_pool(name="p3r", bufs=1) as p3r, \
                 tc.tile_pool(name="ops", bufs=1, space="PSUM") as ops:
                ps_o = []
                for ic in range(4):
                    ps_oc = ops.tile([128, 512], F32, tag=f"o{ic}")
                    ps_o.append(ps_oc)
                for jt in range(JT):
                    if jt < JC:
                        v_src = vcache[:, jt * ROWS:(jt + 1) * ROWS]
                    else:
                        a_t = p3r.tile([128, ROWS], F32, tag="a3")
                        nc.sync.dma_start(
                            out=a_t[:, :],
                            in_=attT[jt * 128:(jt + 1) * 128, :])
                        e_t = p3r.tile([128, ROWS], F32, tag="e3")
                        nc.scalar.activation(e_t[:, :], a_t[:, :], AF.Exp)
                        g_t = p3r.tile([128, ROWS], F32, tag="g3")
                        nc.vector.tensor_scalar(out=g_t[:, :], in0=a_t[:, :],
                                                scalar1=mean_bc[:, 0:1],
                                                scalar2=None, op0=OP.is_gt)
                        v_t = p3r.tile([128, ROWS], F32, tag="v3")
                        nc.vector.tensor_tensor(out=v_t[:, :], in0=g_t[:, :],
                                                in1=e_t[:, :], op=OP.mult)
                        v_src = v_t[:, :]
                    m_t = p3.tile([128, ROWS], U8, tag="m3")
                    nc.sync.dma_start(out=m_t[:, :],
                                      in_=maskT[jt * 128:(jt + 1) * 128, :])
                    mf_t = p3.tile([128, ROWS], F32, tag="mf3")
                    nc.gpsimd.tensor_copy(mf_t[:, :], m_t[:, :])
                    vm_t = p3.tile([128, ROWS], F32, tag="vm3")
                    nc.gpsimd.tensor_tensor(out=vm_t[:, :], in0=v_src,
                                            in1=mf_t[:, :], op=OP.mult)
                    for ic in range(4):
                        nc.tensor.matmul(
                            ps_o[ic][:, :],
                            lhsT=h[:, jt * OUT_F:(jt + 1) * OUT_F],
                            rhs=vm_t[:, ic * 512:(ic + 1) * 512],
                            start=(jt == 0), stop=(jt == JT - 1))
                for ic in range(4):
                    o_t = p3.tile([128, 512], F32, tag="osb")
                    nc.scalar.copy(o_t[:, :], ps_o[ic][:, :])
                    nc.sync.dma_start(out=outT[:, ic * 512:(ic + 1) * 512],
                                      in_=o_t[:, :])
    nc.compile()
    return nc


def kernel(x, attention, W, drop_mask):
    attT = np.ascontiguousarray(attention.T)           # [4096, 16384] f32
    maskT = np.ascontiguousarray(
        drop_mask.astype(np.uint8, copy=False).T)      # [4096, 16384] u8
    xT = np.ascontiguousarray(x.T)                     # [512, 4096]
    wT = np.ascontiguousarray(W.T)                     # [512, 128]

    nc = _build()
    in_maps = []
    for c in range(N_CORES):
        sl = slice(c * ROWS, (c + 1) * ROWS)
        in_maps.append({
            "attT": np.ascontiguousarray(attT[:, sl]),
            "maskT": np.ascontiguousarray(maskT[:, sl]),
            "xT": xT,
            "wT": wT,
        })
    res = run_bass_kernel_spmd(nc, in_maps, core_ids=list(range(N_CORES)))
    global LAST_EXEC_NS
    LAST_EXEC_NS = res.exec_time_ns or res.mean_exec_time_ns
    h_prime = np.concatenate(
        [res.results[c]["outT"].T for c in range(N_CORES)], axis=0)
    out = (h_prime.reshape(4, N, OUT_F).transpose(1, 0, 2)
           .reshape(N, 4 * OUT_F))
    return np.ascontiguousarray(out)


if __name__ == "__main__":
    rng = np.random.default_rng(0)
    x = rng.standard_normal((N, IN_F), dtype=np.float32)
    att = rng.random((CN, N), dtype=np.float32)
    W = (rng.standard_normal((OUT_F, IN_F), dtype=np.float32)
         / np.sqrt(IN_F)).astype(np.float32)
    dm = rng.integers(0, 2, size=(CN, N)).astype(bool)
    out = kernel(x=x, attention=att, W=W, drop_mask=dm)
    print("kernel out", out.shape, out.dtype, float(np.abs(out).max()))
